# revision 20
# baseline (speedup 1.0000x reference)
"""Llama GQA attention layer (B=2, L=2048, D=2048, H=16, KV=4, DH=128) on 8
Trainium2 NeuronCores.

Sharding: batch x KV-head tensor parallel. Cores 0-3 handle batch 0, cores 4-7
batch 1. Within a batch group, core with kv-index k owns KV head k and the 4 Q
heads {4k..4k+3} that attend to it (GQA group size 4), so K/V projections are
computed exactly once fleet-wide. Each core projects its batch's hidden state
against its weight slices, runs RoPE + causal flash attention for its 4 heads,
and multiplies by its 512-column slice of Wo to produce a partial (D, L)
output; the host sums 4 partials per batch.

Device layouts are [feature_dim, position] everywhere so every matmul
contracts over the partition axis; V is PE-transposed in 128-chunks.
RoPE's rotate-half is a fixed 128x128 permutation P applied as one matmul per
projection tile; cos/sin tables are pre-scaled by 128**-0.25 on host so Q.K
picks up the 1/sqrt(DH) softmax scale. Softmax skips max-subtraction (scores
are O(5), exp is safe in fp32). The causal diagonal block uses ragged
(narrower) score/AV matmuls instead of full tiles + mask, with a single
128x128 triangle multiply per diagonal k-tile. The softmax denominator is
tree-summed on the vector engine (fp16) and reduced with a single ones-vector
matmul per q-tile/head.

All DRAM tensors are pre-tiled on host into exactly the SBUF layouts so every
DMA is contiguous per partition; inputs/weights/outputs ride three separate
DMA queues (sync=hidden, scalar=weights, gpsimd=cos/sin+output stores).
"""

import sys

sys.path.insert(0, "/opt/trn_rl_repo")

from contextlib import ExitStack

import numpy as np
import ml_dtypes

import concourse.bass as bass
import concourse.tile as tile
from concourse import bacc
from concourse import mybir
from concourse.bass_utils import run_bass_kernel_spmd

B, L, D = 2, 2048, 2048
H, KV, DH = 16, 4, 128
NCORES = 8
QH = H // KV            # 4 q heads per core (one GQA group)
QD = QH * DH            # 512 q dims per core
THETA = 10000.0

F32 = mybir.dt.float32
F16 = mybir.dt.float16
BF16 = mybir.dt.bfloat16
F32R = mybir.dt.float32r

PC = 512                # projection position-chunk (moving N)
NPC = L // PC           # 4 chunks
NK = D // 128           # 16 contraction tiles / output row tiles
AC = 512                # attention q-tile width
NQT = L // AC           # 4 q-tiles
KT = 128                # k-tile height


def build_program():
    nc = bacc.Bacc("TRN2", target_bir_lowering=False, debug=False)

    hT = nc.dram_tensor("hT", [128, NPC * NK * PC], BF16, kind="ExternalInput")
    wq = nc.dram_tensor("wq", [128, NK * QD], BF16, kind="ExternalInput")
    wk = nc.dram_tensor("wk", [128, NK * DH], BF16, kind="ExternalInput")
    wv = nc.dram_tensor("wv", [128, NK * DH], BF16, kind="ExternalInput")
    wo = nc.dram_tensor("wo", [128, QH * D], BF16, kind="ExternalInput")
    cosb = nc.dram_tensor("cosb", [DH, L], F32, kind="ExternalInput")
    sinb = nc.dram_tensor("sinb", [DH, L], F32, kind="ExternalInput")
    pmat = nc.dram_tensor("pmat", [DH, DH], BF16, kind="ExternalInput")
    idn = nc.dram_tensor("idn", [128, 128], F16, kind="ExternalInput")
    msk = nc.dram_tensor("msk", [KT, KT], F16, kind="ExternalInput")
    onef = nc.dram_tensor("onef", [1, 128], F32R, kind="ExternalInput")
    outp = nc.dram_tensor("outp", [128, NQT * NK * AC], BF16, kind="ExternalOutput")

    with tile.TileContext(nc) as tc, ExitStack() as ctx:
        nc = tc.nc

        # ---------- persistent pools ----------
        const = ctx.enter_context(tc.tile_pool(name="const", bufs=1))
        acts = ctx.enter_context(tc.tile_pool(name="acts", bufs=1))

        ones16_sb = const.tile([128, 1], F16, tag="ones16")
        nc.vector.memset(ones16_sb[:], 1.0)
        onesf_sb = const.tile([1, 128], F32R, tag="onesf")
        wo_sb = const.tile([128, QH * D], BF16, tag="wo")
        msk_sb = const.tile([KT, KT], F16, tag="msk")
        idn_sb = const.tile([128, 128], F16, tag="idn")

        q_sb = acts.tile([128, QH * L], BF16, tag="q")     # head h at cols h*L
        k_sb = acts.tile([128, L], BF16, tag="k")
        vT_sb = acts.tile([128, L], F16, tag="vT")        # k-tile t at cols t*128
        at_sb = acts.tile([128, QH * L], BF16, tag="at")   # attn out, head-major

        # ---------- phase 1: QKV projection + RoPE ----------
        with ExitStack() as p1:
            wpool = p1.enter_context(tc.tile_pool(name="wts", bufs=1))
            hpool = p1.enter_context(tc.tile_pool(name="hid", bufs=2))
            tpool = p1.enter_context(tc.tile_pool(name="tmp", bufs=4))
            vpool = p1.enter_context(tc.tile_pool(name="vstage", bufs=2))
            pps = p1.enter_context(tc.tile_pool(name="pps", bufs=3, space="PSUM"))
            rps = p1.enter_context(tc.tile_pool(name="rps", bufs=2, space="PSUM"))
            tps = p1.enter_context(tc.tile_pool(name="tps", bufs=2, space="PSUM"))

            # weights on the scalar queue, first-needed first; wq split so the
            # first accumulation can chase the DMA stream
            p_sb = wpool.tile([DH, DH], BF16, tag="p")
            nc.scalar.dma_start(p_sb[:], pmat.ap())
            wq_sb = wpool.tile([128, NK * QD], BF16, tag="wq")
            for i in range(NK):
                s = slice(i * QD, (i + 1) * QD)
                nc.scalar.dma_start(wq_sb[:, s], wq.ap()[:, s])
            wk_sb = wpool.tile([128, NK * DH], BF16, tag="wk")
            nc.scalar.dma_start(wk_sb[:], wk.ap())
            wv_sb = wpool.tile([128, NK * DH], BF16, tag="wv")
            nc.scalar.dma_start(wv_sb[:], wv.ap())
            nc.scalar.dma_start(msk_sb[:], msk.ap())
            nc.scalar.dma_start(onesf_sb[:], onef.ap())
            nc.scalar.dma_start(idn_sb[:], idn.ap())
            nc.scalar.dma_start(wo_sb[:], wo.ap())

            # cos/sin per-chunk pieces on the gpsimd queue
            cos_sb = wpool.tile([DH, L], F32, tag="cos")
            sin_sb = wpool.tile([DH, L], F32, tag="sin")
            for c in range(NPC):
                s = slice(c * PC, (c + 1) * PC)
                nc.gpsimd.dma_start(cos_sb[:, s], cosb.ap()[:, s])
                nc.gpsimd.dma_start(sin_sb[:, s], sinb.ap()[:, s])

            def rope_finish(ps, raw, c0, dst):
                rot = rps.tile([128, PC], F32, tag="rot")
                nc.tensor.matmul(rot[:], p_sb[:], raw[:], start=True, stop=True)
                t1 = tpool.tile([128, PC], F32, tag="t1")
                nc.vector.tensor_mul(t1[:], ps[:], cos_sb[:, c0 : c0 + PC])
                t2 = tpool.tile([128, PC], F32, tag="t2")
                nc.vector.tensor_mul(t2[:], rot[:], sin_sb[:, c0 : c0 + PC])
                nc.vector.tensor_add(dst, t1[:], t2[:])

            for pc in range(NPC):
                c0 = pc * PC
                h_t = hpool.tile([128, NK * PC], BF16, tag="h")
                if pc == 0:
                    # per-kt pieces so the first matmuls chase the DMA stream;
                    # contiguous halves ride two queues so arrival matches the
                    # sequential kt consumption order at double bandwidth
                    for kt in range(NK):
                        s = slice(kt * PC, (kt + 1) * PC)
                        eng = nc.sync if kt < NK // 2 else nc.gpsimd
                        eng.dma_start(h_t[:, s], hT.ap()[:, s])
                else:
                    nc.sync.dma_start(
                        h_t[:], hT.ap()[:, pc * NK * PC : (pc + 1) * NK * PC]
                    )

                pending = None  # (ps, raw, dst) awaiting rot matmul + combine

                # Q: four head-tiles, with RoPE (rot deferred one group)
                for mt in range(QH):
                    ps = pps.tile([128, PC], F32, tag="proj")
                    for kt in range(NK):
                        nc.tensor.matmul(
                            ps[:],
                            wq_sb[:, kt * QD + mt * 128 : kt * QD + mt * 128 + 128],
                            h_t[:, kt * PC : (kt + 1) * PC],
                            start=(kt == 0),
                            stop=(kt == NK - 1),
                        )
                    raw = tpool.tile([128, PC], BF16, tag="raw")
                    nc.scalar.activation(
                        raw[:], ps[:], mybir.ActivationFunctionType.Copy
                    )
                    if pending:
                        rope_finish(*pending)
                    pending = (
                        ps,
                        raw,
                        c0,
                        q_sb[:, mt * L + c0 : mt * L + c0 + PC],
                    )

                # V: one head-tile, stage as bf16 for PE-transpose
                ps = pps.tile([128, PC], F32, tag="proj")
                for kt in range(NK):
                    nc.tensor.matmul(
                        ps[:],
                        wv_sb[:, kt * DH : (kt + 1) * DH],
                        h_t[:, kt * PC : (kt + 1) * PC],
                        start=(kt == 0),
                        stop=(kt == NK - 1),
                    )
                vst = vpool.tile([128, PC], F16, tag="vst")
                nc.vector.tensor_copy(vst[:], ps[:])
                if pending:
                    rope_finish(*pending)
                    pending = None

                # K: one head-tile, with RoPE
                ps = pps.tile([128, PC], F32, tag="proj")
                for kt in range(NK):
                    nc.tensor.matmul(
                        ps[:],
                        wk_sb[:, kt * DH : (kt + 1) * DH],
                        h_t[:, kt * PC : (kt + 1) * PC],
                        start=(kt == 0),
                        stop=(kt == NK - 1),
                    )
                raw = tpool.tile([128, PC], BF16, tag="raw")
                nc.scalar.activation(raw[:], ps[:], mybir.ActivationFunctionType.Copy)

                # V transposes (vst copy had the K-group matmuls to finish)
                for tt in range(PC // 128):
                    tp = tps.tile([128, 128], F16, tag="tp")
                    nc.tensor.transpose(
                        tp[:], vst[:, tt * 128 : (tt + 1) * 128], idn_sb[:]
                    )
                    nc.vector.tensor_copy(
                        vT_sb[:, c0 + tt * 128 : c0 + (tt + 1) * 128], tp[:]
                    )

                rope_finish(ps, raw, c0, k_sb[:, c0 : c0 + PC])

        # ---------- phase 2: flash attention + inline Wo ----------
        with ExitStack() as p2:
            epool = p2.enter_context(tc.tile_pool(name="exp", bufs=10))
            espool = p2.enter_context(tc.tile_pool(name="esum", bufs=3))
            spool = p2.enter_context(tc.tile_pool(name="small", bufs=4))
            opool = p2.enter_context(tc.tile_pool(name="wob", bufs=4))
            scps = p2.enter_context(tc.tile_pool(name="scps", bufs=2, space="PSUM"))
            lps = p2.enter_context(tc.tile_pool(name="lps", bufs=1, space="PSUM"))
            ops = p2.enter_context(tc.tile_pool(name="ops", bufs=2, space="PSUM"))
            bps = p2.enter_context(tc.tile_pool(name="bps", bufs=1, space="PSUM"))

            def wo_group(src_qt, dt):
                """One 128-row slice of the Wo projection for position block
                src_qt; interleaved into the exp-paced attention stream."""
                sq0 = src_qt * AC
                ps = ops.tile([128, AC], F32, tag="o")
                for et in range(QH):
                    nc.tensor.matmul(
                        ps[:],
                        wo_sb[:, et * D + dt * 128 : et * D + (dt + 1) * 128],
                        at_sb[:, et * L + sq0 : et * L + sq0 + AC],
                        start=(et == 0),
                        stop=(et == QH - 1),
                    )
                ob = opool.tile([128, AC], BF16, tag="ob")
                if dt % 8 < 3:
                    nc.scalar.activation(
                        ob[:], ps[:], mybir.ActivationFunctionType.Copy
                    )
                else:
                    nc.vector.tensor_copy(ob[:], ps[:])
                nc.sync.dma_start(
                    outp.ap()[
                        :, (src_qt * NK + dt) * AC : (src_qt * NK + dt + 1) * AC
                    ],
                    ob[:],
                )

            def attention(qt, h):
                q0 = qt * AC
                nk = (q0 + AC) // KT
                q_ap = q_sb[:, h * L + q0 : h * L + q0 + AC]

                # per k-tile: query offset (ragged on the causal diagonal)
                def qoff(kt):
                    return max(0, KT * (kt - (nk - 4)))

                infos = []  # (e_tile, col_off, q_off, width, kt)
                for j in range(nk // 2):
                    kt0, kt1 = 2 * j, 2 * j + 1
                    o0, o1 = qoff(kt0), qoff(kt1)
                    w0, w1 = AC - o0, AC - o1
                    same_bank = w0 < AC  # pack both into bank A
                    c1 = w0 if same_bank else AC
                    sc = scps.tile([KT, 2 * AC], F32, tag="sc")
                    # per-PSUM-bank accumulation groups: score matmul plus, on
                    # diagonal tiles, a -30000 upper-triangle bias matmul that
                    # exp maps to an exact zero (replaces a vector mask mul)
                    banks = [[], []] if not same_bank else [[], None]
                    for coff, o, w, kt in ((0, o0, w0, kt0), (c1, o1, w1, kt1)):
                        grp = banks[0] if (same_bank or coff == 0) else banks[1]
                        grp.append(
                            (
                                sc[:, coff : coff + w],
                                k_sb[:, kt * KT : (kt + 1) * KT],
                                q_ap[:, o : o + w],
                            )
                        )
                        if kt >= nk - 4:
                            grp.append(
                                (sc[:, coff : coff + KT], msk_sb[:], idn_sb[:])
                            )
                    for grp in banks:
                        if not grp:
                            continue
                        for gi, (out_ap, lhsT, rhs) in enumerate(grp):
                            nc.tensor.matmul(
                                out_ap,
                                lhsT,
                                rhs,
                                start=(gi == 0),
                                stop=(gi == len(grp) - 1),
                                skip_group_check=True,
                            )
                    e = epool.tile([KT, 2 * AC], F16, tag="e")
                    nc.scalar.activation(
                        e[:, 0 : c1 + w1],
                        sc[:, 0 : c1 + w1],
                        mybir.ActivationFunctionType.Exp,
                    )
                    infos.append((e, 0, o0, w0, kt0))
                    infos.append((e, c1, o1, w1, kt1))

                # denominator accumulated on vector (pure fp16 path)
                esum = espool.tile([128, AC], F16, tag="esum")
                (e0, _, _, _, _), (_, c1, o1, w1, _) = infos[0], infos[1]
                if o1 == 0:
                    nc.vector.tensor_add(
                        esum[:], e0[:, 0:AC], e0[:, c1 : c1 + AC]
                    )
                else:
                    nc.vector.tensor_copy(esum[:, 0:o1], e0[:, 0:o1])
                    nc.vector.tensor_add(
                        esum[:, o1:AC], e0[:, o1:AC], e0[:, c1 : c1 + w1]
                    )
                for e_, coff, o, w, kt in infos[2:]:
                    nc.vector.tensor_add(
                        esum[:, o : o + w],
                        esum[:, o : o + w],
                        e_[:, coff : coff + w],
                    )

                # A @ V with ragged diagonal
                o_ps = ops.tile([128, AC], F32, tag="o")
                for idx, (e_, coff, o, w, kt) in enumerate(infos):
                    nc.tensor.matmul(
                        o_ps[:, o : o + w],
                        vT_sb[:, kt * KT : (kt + 1) * KT],
                        e_[:, coff : coff + w],
                        start=(idx == 0),
                        stop=(idx == nk - 1),
                        skip_group_check=True,
                    )

                # l = colsum(esum); rec = 1/l broadcast to 128 partitions
                l_ps = lps.tile([1, AC], F32, tag="l")
                nc.tensor.matmul(
                    l_ps[:], ones16_sb[:, 0:1], esum[:], start=True, stop=True
                )
                l_sb = spool.tile([1, AC], F32R, tag="lsb")
                nc.vector.tensor_copy(l_sb[:], l_ps[:])
                bc = bps.tile([128, AC], F32, tag="bc")
                nc.tensor.matmul(
                    bc[:], onesf_sb[:, :], l_sb[:], start=True, stop=True
                )
                rec = spool.tile([128, AC], F32, tag="rec")
                nc.vector.reciprocal_approx_fast(rec[:], bc[:])
                nc.vector.tensor_mul(
                    at_sb[:, h * L + q0 : h * L + q0 + AC], o_ps[:], rec[:]
                )

            # Wo(qt-1) groups are interleaved into attention(qt) so the
            # tensor engine fills its exp-wait gaps with dense Wo matmuls
            for qt in range(NQT):
                for h in range(QH):
                    attention(qt, h)
                    if qt > 0:
                        for dt in range(4 * h, 4 * h + 4):
                            wo_group(qt - 1, dt)
            for dt in range(NK):
                wo_group(NQT - 1, dt)

    nc.compile()
    return nc


_NC = None


def _tables():
    inv_freq = 1.0 / (THETA ** (np.arange(0, DH, 2, dtype=np.float64) / DH))
    pos = np.arange(L, dtype=np.float64)
    freq = pos[:, None] * inv_freq[None, :]
    emb = np.concatenate([freq, freq], axis=1)          # (L, DH)
    s = 128.0 ** -0.25
    cos_t = (np.cos(emb).T * s).astype(np.float32)      # (DH, L)
    sin_t = (np.sin(emb).T * s).astype(np.float32)

    pm = np.zeros((DH, DH), np.float32)
    i = np.arange(DH // 2)
    pm[DH // 2 + i, i] = -1.0                           # lhsT for rot = P @ x
    pm[i, DH // 2 + i] = 1.0

    idn = np.eye(128, dtype=np.float16)

    jj = np.arange(KT)[:, None]
    kk = np.arange(KT)[None, :]
    mk = np.where(kk > jj, -30000.0, 0.0).astype(np.float16)
    return cos_t, sin_t, pm.astype(ml_dtypes.bfloat16), idn, mk


def _tile_rows(a):
    """[R, M] -> [128, (R//128)*M] with row-tile t at cols t*M."""
    r = a.shape[0] // 128
    return np.ascontiguousarray(
        a.reshape(r, 128, a.shape[1]).transpose(1, 0, 2).reshape(128, r * a.shape[1])
    )


def _in_maps(hidden_state, Wq, Wk, Wv, Wo):
    cos_t, sin_t, pm, idn, mk = _tables()
    bf = ml_dtypes.bfloat16
    maps = []
    for c in range(NCORES):
        b, kv = divmod(c, KV)
        qs = slice(kv * QD, (kv + 1) * QD)
        ks = slice(kv * DH, (kv + 1) * DH)
        ht = hidden_state[b].T                              # (D, L)
        h_tiled = np.ascontiguousarray(
            ht.reshape(NK, 128, NPC, PC).transpose(1, 2, 0, 3).reshape(128, NPC * NK * PC)
        ).astype(bf)
        maps.append(
            {
                "hT": h_tiled,
                "wq": _tile_rows(Wq[qs].T.astype(np.float32)).astype(bf),
                "wk": _tile_rows(Wk[ks].T.astype(np.float32)).astype(bf),
                "wv": _tile_rows(Wv[ks].T.astype(np.float32)).astype(bf),
                "wo": _tile_rows(Wo[:, qs].T.astype(np.float32)).astype(bf),
                "cosb": cos_t,
                "sinb": sin_t,
                "pmat": pm,
                "idn": idn,
                "msk": mk,
                "onef": np.ones((1, 128), np.float32),
            }
        )
    return maps


def kernel(hidden_state, attention_mask, Wq, Wk, Wv, Wo):
    global _NC
    if _NC is None:
        _NC = build_program()
    nc = _NC

    hidden_state = np.asarray(hidden_state, np.float32)
    Wq = np.asarray(Wq, np.float32)
    Wk = np.asarray(Wk, np.float32)
    Wv = np.asarray(Wv, np.float32)
    Wo = np.asarray(Wo, np.float32)

    maps = _in_maps(hidden_state, Wq, Wk, Wv, Wo)
    res = run_bass_kernel_spmd(nc, maps, core_ids=list(range(NCORES)))

    out = np.zeros((B, D, L), np.float32)
    for c in range(NCORES):
        x = np.asarray(res.results[c]["outp"], dtype=np.float32)
        x = x.reshape(128, NQT, NK, AC).transpose(2, 0, 1, 3).reshape(D, L)
        out[c // KV] += x
    return np.ascontiguousarray(out.transpose(0, 2, 1)).astype(np.float32)
